# revision 34
# baseline (speedup 1.0000x reference)
"""Trainium2 Bass kernel for nn_Attention (LayerNorm + MHA + rel-pos-bias + out proj).

Sharding: 16 heads / 8 cores = 2 consecutive heads per core (tensor parallel);
every core processes all 4 batches. Each core computes the partial
out-projection for its 2 heads; the host sums the 8 partials and adds b_out.

Key structure (per core, heads h0=2c, h1=2c+1):
  - Host pre-transposes x -> xT [D, TOK] (bf16) and folds the LayerNorm
    mean-removal into the QKV weights: W'' = (gamma*W) - colsum(gamma*W)/D,
    so  qkv = rs[t] * (x @ W'')  with rs = 1/sqrt(var+eps) the only
    x-dependent LN quantity (computed on device via bn_stats).
  - Q^T = W''_q^T xT  (PSUM), evicted * rs_row (replicated via DRAM bounce)
  - K^T = W''_k^T xT  evicted as plain copy; rs_k * 1/sqrt(dh) is folded
    into the exp's per-partition scale instead.
  - V^T = W''_v^T xT  evicted * rs_row, then PE-transposed into V[k, d]
    tiles with an extra ones column (row 64/129) for softmax denominators.
  - S^T = K_h Q_h^T per kt; one exp per kt covers both heads
    ([128, 2, 512], scale = rs_k/sqrt(dh)); P *= exp(bias) (host pre-exp,
    bf16), alternating DVE/Pool.
  - O'^T = V'_h^T P^T accumulated over kt -> [65, 512]; row 64 = denom.
  - OT = O^T * (1/denom) (bounced through DRAM for partition broadcast).
  - Y = OT^T W_out, one 1024-col matmul per token tile, interleaved into
    the last attention pass; partial output summed on host (f32 via bf16).

All matmul operands bf16, accumulation f32 in PSUM. PSUM budget 8 banks:
ps-pool 3x[128, 1024]f32 (QKV accum / scores / out-proj), po-pool
2x[65, 512]f32 (attnV accumulators, also transposition scratch).
"""

import os
import sys

for _p in ("/opt/trn_rl_repo",):
    if os.path.isdir(_p) and _p not in sys.path:
        sys.path.insert(0, _p)

import numpy as np
import ml_dtypes

import bass_rust
import concourse.bass as bass
import concourse.mybir as mybir
import concourse.tile as tile
from concourse.bass_utils import run_bass_kernel_spmd
from concourse.masks import make_identity

BF16 = mybir.dt.bfloat16
F32 = mybir.dt.float32
NPBF16 = ml_dtypes.bfloat16
AF = mybir.ActivationFunctionType
ALU = mybir.AluOpType

B, N, D = 4, 2048, 1024
HEADS, HD = 16, 64
P = 128
NCORES = 8
HPC = HEADS // NCORES          # heads per core = 2
TOK = B * N                    # 8192
QB = 512                       # q block for attention phase
NQB = N // QB                  # 4
NKT = N // P                   # 16 key tiles
DC = D // P                    # 8 model-dim chunks
GRP = 1024                     # token group for QKV matmuls
NGRP = N // GRP                # 2 groups per batch
EPS = 1e-5
SCALE = HD ** -0.5


def _split_waits(nc, maxw=1):
    """This walrus build rejects instructions with more than one sync wait;
    move excess waits onto preceding same-engine NoOps (1 wait each)."""
    n_new = 0
    for bb in nc.main_func.blocks:
        out, changed = [], False
        for ins in bb.instructions:
            si = ins.sync_info
            if si is not None and si.on_wait and len(si.on_wait) > maxw:
                ow = list(si.on_wait)
                head, tail = ow[:-maxw], ow[-maxw:]
                for i, w in enumerate(head):
                    nop = mybir.InstNoOp(name=f"waitsplit_{ins.name}_{i}")
                    nop.engine = ins.engine
                    nop.sync_info = bass_rust.SyncInfo(on_wait=[w], on_update=[])
                    out.append(nop)
                    n_new += 1
                si.on_wait = tail
                changed = True
            out.append(ins)
        if changed:
            bb.instructions = out
    return n_new


def _build_graph(use_qkv_bias):
    nc = bass.Bass(target_bir_lowering=False)

    x = nc.declare_dram_parameter("x", [TOK, D], BF16, isOutput=False)
    xt = nc.declare_dram_parameter("xt", [D, TOK], BF16, isOutput=False)
    wq = nc.declare_dram_parameter("wq", [D, P], BF16, isOutput=False)
    wk = nc.declare_dram_parameter("wk", [D, P], BF16, isOutput=False)
    wv = nc.declare_dram_parameter("wv", [D, P], BF16, isOutput=False)
    wo = nc.declare_dram_parameter("wo", [P, D], BF16, isOutput=False)
    ebt = nc.declare_dram_parameter("ebt", [HPC, NKT, NQB, P, QB], BF16, isOutput=False)
    if use_qkv_bias:
        qbq = nc.declare_dram_parameter("qbq", [P], F32, isOutput=False)
        qbk = nc.declare_dram_parameter("qbk", [P], F32, isOutput=False)
        qbv = nc.declare_dram_parameter("qbv", [P], F32, isOutput=False)
    y = nc.declare_dram_parameter("out", [TOK, D], BF16, isOutput=True)

    rs_dram = nc.dram_tensor("rs_scratch", [B, N], F32)
    den_dram = nc.dram_tensor("den_scratch", [B, NQB, HPC, QB], F32)

    with tile.TileContext(nc) as tc:
        with tc.tile_pool(name="singles", bufs=1) as singles, \
             tc.tile_pool(name="pa_x", bufs=6) as pa_x, \
             tc.tile_pool(name="pa_xt", bufs=11) as pa_xt, \
             tc.tile_pool(name="pa_small", bufs=6) as pas, \
             tc.tile_pool(name="pa_rep", bufs=2) as prep, \
             tc.tile_pool(name="pa_vt", bufs=2) as pvt, \
             tc.tile_pool(name="pb_ebt", bufs=20) as pbe, \
             tc.tile_pool(name="pb_p", bufs=6) as pbp, \
             tc.tile_pool(name="pb_small", bufs=2) as pbs, \
             tc.tile_pool(name="pc_y", bufs=2) as pcy, \
             tc.tile_pool(name="ps", bufs=2, space="PSUM") as psA, \
             tc.tile_pool(name="po", bufs=4, space="PSUM") as psO:

            # ---- persistent SBUF state ----
            wq_sb = singles.tile([P, DC, P], BF16, tag="wq")
            nc.sync.dma_start(wq_sb[:], wq.ap().rearrange("(c p) m -> p c m", p=P))
            wk_sb = singles.tile([P, DC, P], BF16, tag="wk")
            nc.sync.dma_start(wk_sb[:], wk.ap().rearrange("(c p) m -> p c m", p=P))
            wv_sb = singles.tile([P, DC, P], BF16, tag="wv")
            nc.sync.dma_start(wv_sb[:], wv.ap().rearrange("(c p) m -> p c m", p=P))
            wo_sb = singles.tile([P, D], BF16, tag="wo")
            nc.sync.dma_start(wo_sb[:], wo.ap())

            eps_sb = singles.tile([P, 1], F32, tag="eps")
            nc.vector.memset(eps_sb[:], EPS)
            id_bf = singles.tile([P, P], BF16, tag="idbf")
            make_identity(nc, id_bf[:])
            id_f32 = singles.tile([P, P], F32, tag="idf32")
            make_identity(nc, id_f32[:])

            if use_qkv_bias:
                qbq_sb = singles.tile([P, 1], F32, tag="qbq")
                nc.sync.dma_start(qbq_sb[:], qbq.ap()[:, None])
                qbk_sb = singles.tile([P, 1], F32, tag="qbk")
                nc.sync.dma_start(qbk_sb[:], qbk.ap()[:, None])
                qbv_sb = singles.tile([P, 1], F32, tag="qbv")
                nc.sync.dma_start(qbv_sb[:], qbv.ap()[:, None])

            QT = [singles.tile([P, N], BF16, tag=f"QT{b}", name=f"QT{b}") for b in range(B)]
            KT = [singles.tile([P, N], BF16, tag=f"KT{b}", name=f"KT{b}") for b in range(B)]
            # V'[k, kt, h, :]: cols 0:64 head h data, col 64 ones (denom row)
            VV = [singles.tile([P, NKT, HPC, HD + 1], BF16, tag=f"V{b}", name=f"V{b}")
                  for b in range(B)]
            OT = [singles.tile([P, N], BF16, tag=f"OT{b}", name=f"OT{b}") for b in range(B)]
            rs_all = [singles.tile([P, NKT], F32, tag=f"rs{b}", name=f"rs{b}") for b in range(B)]
            rs_sc = [singles.tile([P, NKT], F32, tag=f"rssc{b}", name=f"rssc{b}") for b in range(B)]
            for b in range(B):
                nc.vector.memset(VV[b][:, :, :, HD:HD + 1], 1.0)

            def phase_stats(b):
                """Per-token LayerNorm 1/sigma for batch b -> rs_all[b],
                rs_sc[b]. The DRAM bounce for the row-major copy is emitted
                inside phase_qkv (after the K matmuls) so the PE never heads
                the program with a stats-gated transpose."""
                for t in range(NKT):
                    r = b * N + t * P
                    xt2 = pa_x.tile([P, D], BF16, tag="xt")
                    nc.sync.dma_start(xt2[:], x.ap()[r:r + P, :])
                    stats = pas.tile([P, 2, 6], F32, tag="stats")
                    xt3 = xt2[:].rearrange("p (s f) -> p s f", s=2)
                    nc.vector.bn_stats(stats[:, 0, :], xt3[:, 0, :])
                    nc.vector.bn_stats(stats[:, 1, :], xt3[:, 1, :])
                    mv = pas.tile([P, 2], F32, tag="mv")
                    nc.vector.bn_aggr(mv[:], stats[:])
                    std = pas.tile([P, 1], F32, tag="std")
                    nc.scalar.activation(std[:], mv[:, 1:2], AF.Sqrt,
                                         bias=eps_sb[:])
                    nc.vector.reciprocal(rs_all[b][:, t:t + 1], std[:])
                nc.vector.tensor_scalar(rs_sc[b][:], rs_all[b][:], SCALE, None,
                                        op0=ALU.mult)

            def rs_bounce(b):
                """Transpose rs_all[b] [128, 16] -> token-major DRAM row."""
                pst = psO.tile([NKT, P], F32, tag="po", name=f"pst{b}")
                nc.tensor.transpose(pst[:], rs_all[b][:], id_f32[:])
                rst = pas.tile([NKT, P], F32, tag="rst")
                nc.scalar.copy(rst[:], pst[:])
                nc.scalar.dma_start(
                    rs_dram.ap()[b].rearrange("(t p) -> t p", p=P), rst[:])

            def phase_qkv(b):
                """QKV projections for batch b from pre-transposed x."""
                rs_rep = prep.tile([P, N], F32, tag="rsrep", name=f"rsrep{b}")
                for g in range(NGRP):
                    gsl = slice(g * GRP, (g + 1) * GRP)
                    xtc = []
                    for c in range(DC):
                        xc = pa_xt.tile([P, GRP], BF16, tag="xtc")
                        nc.sync.dma_start(
                            xc[:], xt.ap()[c * P:(c + 1) * P,
                                           b * N + g * GRP:b * N + (g + 1) * GRP])
                        xtc.append(xc[:])
                    # Q^T (matmul outputs are per-half: one PSUM bank each)
                    psq = psA.tile([P, GRP], F32, tag="ps", name=f"psq{b}_{g}")
                    for hf in range(GRP // QB):
                        hsl = slice(hf * QB, (hf + 1) * QB)
                        for c in range(DC):
                            nc.tensor.matmul(psq[:, hsl], wq_sb[:, c, :],
                                             xtc[c][:, hsl],
                                             start=(c == 0), stop=(c == DC - 1))
                    # K^T matmuls come before the Q eviction so the rs
                    # bounce (PE transpose gated on stats) sits behind ~9us
                    # of QK matmul work in PE program order.
                    psk = psA.tile([P, GRP], F32, tag="ps", name=f"psk{b}_{g}")
                    for hf in range(GRP // QB):
                        hsl = slice(hf * QB, (hf + 1) * QB)
                        for c in range(DC):
                            nc.tensor.matmul(psk[:, hsl], wk_sb[:, c, :],
                                             xtc[c][:, hsl],
                                             start=(c == 0), stop=(c == DC - 1))
                    if g == 0:
                        rs_bounce(b)
                        rep_src = bass.AP(tensor=rs_dram, offset=b * N,
                                          ap=[[0, P], [1, N]])
                        nc.scalar.dma_start(out=rs_rep[:], in_=rep_src)
                    nc.vector.tensor_tensor(QT[b][:, gsl], psq[:], rs_rep[:, gsl],
                                            ALU.mult)
                    if use_qkv_bias:
                        nc.vector.tensor_scalar_add(QT[b][:, gsl], QT[b][:, gsl],
                                                    qbq_sb[:])
                    # K^T eviction (rs_k folded into exp scale)
                    if use_qkv_bias:
                        # K must carry its bias pre-exp-scale: K = K_raw*rs + c
                        # but rs is folded into exp; divide bias out instead is
                        # wrong — fall back to explicit rs application.
                        kf = pas.tile([P, GRP], F32, tag="ktmp")
                        nc.vector.tensor_tensor(kf[:], psk[:], rs_rep[:, gsl],
                                                ALU.mult)
                        nc.vector.tensor_scalar_add(KT[b][:, gsl], kf[:], qbk_sb[:])
                    else:
                        nc.scalar.copy(KT[b][:, gsl], psk[:])
                    # V^T, then PE-transpose into V[k, d] tiles
                    psv = psA.tile([P, GRP], F32, tag="ps", name=f"psv{b}_{g}")
                    for hf in range(GRP // QB):
                        hsl = slice(hf * QB, (hf + 1) * QB)
                        for c in range(DC):
                            nc.tensor.matmul(psv[:, hsl], wv_sb[:, c, :],
                                             xtc[c][:, hsl],
                                             start=(c == 0), stop=(c == DC - 1))
                    vt = pvt.tile([P, GRP], BF16, tag="vt")
                    nc.vector.tensor_tensor(vt[:], psv[:], rs_rep[:, gsl],
                                            ALU.mult)
                    if use_qkv_bias:
                        nc.vector.tensor_scalar_add(vt[:], vt[:], qbv_sb[:])
                    for j in range(GRP // P):
                        kt = g * (GRP // P) + j
                        ptr = psO.tile([P, P], BF16, tag="po",
                                       name=f"ptr{b}_{kt}")
                        nc.tensor.transpose(ptr[:], vt[:, j * P:(j + 1) * P],
                                            id_bf[:])
                        ptr2 = ptr[:].rearrange("p (s d) -> p s d", s=HPC)
                        nc.scalar.copy(VV[b][:, kt, :, 0:HD], ptr2[:])

            def phase_b_qb(qb, with_c_for=None):
                """Attention for one q block across all batches; the 16 ebt
                tiles for this q block are loaded once and reused 4x.
                with_c_for(b) returns a list of (batch, token-tile) C-work
                items to spread across this (qb, b) kt loop."""
                qsl = slice(qb * QB, (qb + 1) * QB)
                ebts = []
                for kt in range(NKT):
                    ebt2 = pbe.tile([P, HPC, QB], BF16, tag="ebt",
                                    name=f"ebt_{qb}_{kt}")
                    nc.gpsimd.dma_start(
                        ebt2[:],
                        ebt.ap()[:, kt, qb].rearrange("h p q -> p h q"))
                    ebts.append(ebt2)
                for b in range(B):
                    clist = with_c_for(b) if with_c_for else []
                    # C tiles start at kt>=6: the deferred finish of the
                    # previous batch holds two psO slots until ~kt 4-5
                    ctiles = {6 + (i * (NKT - 6)) // len(clist): ct
                              for i, ct in enumerate(clist)} if clist else {}
                    pso0 = psO.tile([HD + 1, QB], F32, tag="po",
                                    name=f"psO0_{b}_{qb}")
                    pso1 = psO.tile([HD + 1, QB], F32, tag="po",
                                    name=f"psO1_{b}_{qb}")
                    p0s = []
                    for kt in range(NKT):
                        ksl = slice(kt * P, (kt + 1) * P)
                        pss = psA.tile([P, HPC, QB], F32, tag="ps",
                                       name=f"psS{b}_{qb}_{kt}")
                        nc.tensor.matmul(pss[:, 0, :], KT[b][0:HD, ksl],
                                         QT[b][0:HD, qsl], start=True, stop=True)
                        nc.tensor.matmul(pss[:, 1, :], KT[b][HD:P, ksl],
                                         QT[b][HD:P, qsl], start=True, stop=True)
                        p0 = pbp.tile([P, HPC, QB], BF16, tag="p0")
                        # rs_k (and 1/sqrt(dh)) ride the exp's per-partition
                        # scale; in the bias fallback K already carries rs.
                        sc = SCALE if use_qkv_bias else rs_sc[b][:, kt:kt + 1]
                        nc.scalar.activation(p0[:], pss[:], AF.Exp, scale=sc)
                        nc.vector.tensor_tensor(p0[:], p0[:], ebts[kt][:],
                                                ALU.mult)
                        p0s.append(p0)
                        if kt == 1 and pending:
                            pending.pop(0)()
                        if kt > 1:
                            pp = p0s[kt - 2]
                            nc.tensor.matmul(pso0[:], VV[b][:, kt - 2, 0, :],
                                             pp[:, 0, :],
                                             start=(kt == 2), stop=False)
                            nc.tensor.matmul(pso1[:], VV[b][:, kt - 2, 1, :],
                                             pp[:, 1, :],
                                             start=(kt == 2), stop=False)
                        if ctiles and kt in ctiles:
                            _phase_c_tile(*ctiles[kt])
                    for kt in (NKT - 2, NKT - 1):
                        pp = p0s[kt]
                        nc.tensor.matmul(pso0[:], VV[b][:, kt, 0, :],
                                         pp[:, 0, :], start=False, stop=(kt == NKT - 1))
                        nc.tensor.matmul(pso1[:], VV[b][:, kt, 1, :],
                                         pp[:, 1, :], start=False, stop=(kt == NKT - 1))
                    pending.append(_batch_finish(b, qb, pso0, pso1))

            def _batch_finish(b, qb, pso0, pso1):
                """Deferred: denominators -> reciprocal -> DRAM bounce -> OT
                eviction. Emitted a couple of iterations into the NEXT batch
                so it pipelines under the exp/mult stream instead of stalling
                the DVE/ACT queues at the batch boundary."""
                qsl = slice(qb * QB, (qb + 1) * QB)

                def run():
                    rec0 = pbs.tile([1, QB], F32, tag="rec0")
                    nc.scalar.copy(rec0[:], pso0[HD:HD + 1, :])
                    nc.gpsimd.dma_start(den_dram.ap()[b, qb, 0:1, :], rec0[:])
                    rec1 = pbs.tile([1, QB], F32, tag="rec1")
                    nc.scalar.copy(rec1[:], pso1[HD:HD + 1, :])
                    nc.gpsimd.dma_start(den_dram.ap()[b, qb, 1:2, :], rec1[:])
                    rb = pbs.tile([P, QB], F32, tag="rb")
                    for h in range(HPC):
                        rb_src = bass.AP(tensor=den_dram,
                                         offset=((b * NQB + qb) * HPC + h) * QB,
                                         ap=[[0, HD], [1, QB]])
                        nc.gpsimd.dma_start(out=rb[h * HD:(h + 1) * HD, :],
                                            in_=rb_src)
                    nc.vector.reciprocal(rb[:], rb[:])
                    nc.vector.tensor_tensor(OT[b][0:HD, qsl], pso0[0:HD, :],
                                            rb[0:HD, :], ALU.mult)
                    nc.vector.tensor_tensor(OT[b][HD:P, qsl], pso1[0:HD, :],
                                            rb[HD:P, :], ALU.mult)
                return run

            def _phase_c_tile(b, t):
                """One token tile of the partial out-projection for batch b."""
                tsl = slice(t * P, (t + 1) * P)
                yt = pcy.tile([P, D], BF16, tag="yt")
                for hf in range(2):
                    hsl = slice(hf * QB, (hf + 1) * QB)
                    psy = psO.tile([P, QB], F32, tag="po", name=f"psy{b}_{t}_{hf}")
                    nc.tensor.matmul(psy[:], OT[b][:, tsl], wo_sb[:, hsl],
                                     start=True, stop=True)
                    nc.vector.tensor_copy(yt[:, hsl], psy[:])
                nc.gpsimd.dma_start(
                    y.ap()[b * N + t * P:b * N + (t + 1) * P, :], yt[:])

            pending = []

            for b in range(B):
                phase_stats(b)
                phase_qkv(b)

            def c_work(qb):
                def inner(b):
                    # a C tile t only needs OT[:, t-block] == q-block t//4:
                    # during (qb, b) run batch b's (qb-1) tiles, plus on the
                    # last q block the previous batch's (qb=3) tiles.
                    tiles = []
                    if qb >= 1:
                        tiles += [(b, (qb - 1) * 4 + j) for j in range(4)]
                    if qb == NQB - 1 and b >= 1:
                        tiles += [(b - 1, (NQB - 1) * 4 + j) for j in range(4)]
                    return tiles
                return inner

            for qb in range(NQB):
                phase_b_qb(qb, with_c_for=c_work(qb))
            while pending:
                pending.pop(0)()
            for j in range(4):
                _phase_c_tile(B - 1, (NQB - 1) * 4 + j)

    _split_waits(nc)
    return nc


_GRAPH_CACHE = {}


def _get_graph(use_qkv_bias):
    if use_qkv_bias not in _GRAPH_CACHE:
        _GRAPH_CACHE[use_qkv_bias] = _build_graph(use_qkv_bias)
    return _GRAPH_CACHE[use_qkv_bias]


def kernel(x, relative_position_bias, w_qkv, w_out, b_out, ln_gamma, ln_beta,
           _run_kwargs=None):
    x = np.asarray(x, dtype=np.float32)
    bias = np.asarray(relative_position_bias, dtype=np.float32)
    w_qkv = np.asarray(w_qkv, dtype=np.float32)
    w_out = np.asarray(w_out, dtype=np.float32)
    b_out = np.asarray(b_out, dtype=np.float32)
    ln_gamma = np.asarray(ln_gamma, dtype=np.float32)
    ln_beta = np.asarray(ln_beta, dtype=np.float32)

    # fold LN affine into the QKV projection: with W' = gamma*W and
    # W'' = W' - colsum(W')/D,  xn @ W = rs * (x @ W'') + beta @ W
    w = w_qkv * ln_gamma[:, None]                       # [D, 3D]
    w = w - np.sum(w, axis=0, keepdims=True) / D        # fold mean removal
    qkv_bias = ln_beta @ w_qkv                          # [3D]
    use_qkv_bias = bool(np.any(qkv_bias != 0.0))

    x2 = np.ascontiguousarray(x.reshape(TOK, D))
    x_bf = x2.astype(NPBF16)
    xt_bf = np.ascontiguousarray(x2.T.astype(NPBF16))
    eb = np.exp(bias)                                   # [16, N, N]

    in_maps = []
    for c in range(NCORES):
        h0 = HPC * c
        csl = slice(h0 * HD, (h0 + HPC) * HD)
        m = {
            "x": x_bf,
            "xt": xt_bf,
            "wq": np.ascontiguousarray(w[:, csl]).astype(NPBF16),
            "wk": np.ascontiguousarray(w[:, D + h0 * HD:D + (h0 + HPC) * HD]).astype(NPBF16),
            "wv": np.ascontiguousarray(w[:, 2 * D + h0 * HD:2 * D + (h0 + HPC) * HD]).astype(NPBF16),
            "wo": np.ascontiguousarray(w_out[csl, :]).astype(NPBF16),
            # [h, kt, qb, p(k-within-chunk), q] with each [p, q] tile contiguous
            "ebt": np.ascontiguousarray(
                eb[h0:h0 + HPC].transpose(0, 2, 1)          # [h, k, q]
                .reshape(HPC, NKT, P, NQB, QB)
                .transpose(0, 1, 3, 2, 4)).astype(NPBF16),
        }
        if use_qkv_bias:
            m["qbq"] = np.ascontiguousarray(qkv_bias[csl])
            m["qbk"] = np.ascontiguousarray(qkv_bias[D + h0 * HD:D + (h0 + HPC) * HD])
            m["qbv"] = np.ascontiguousarray(qkv_bias[2 * D + h0 * HD:2 * D + (h0 + HPC) * HD])
        in_maps.append(m)

    nc = _get_graph(use_qkv_bias)
    kwargs = dict(_run_kwargs or {})
    res = run_bass_kernel_spmd(nc, in_maps, core_ids=list(range(NCORES)), **kwargs)

    acc = np.zeros((TOK, D), dtype=np.float32)
    for c in range(NCORES):
        acc += np.asarray(res.results[c]["out"], dtype=np.float32)
    out = acc + b_out[None, :]
    if _run_kwargs is not None:
        kernel.last_result = res
    return out.reshape(B, N, D).astype(np.float32)
